# revision 1
# baseline (speedup 1.0000x reference)
"""Trainium2 Bass kernel for nn_AudioEvent: oscillator bank + FFT-filtered noise synth.

Sharding: data-parallel over batch (B=8) -> one batch element per NeuronCore.

Per-core algorithm (all heavy compute on device):
  - phase(t) = freq_rows @ V  (V = cumulative linear-interp weights; cumsum(interp(f))
    is exactly a matmul since interp is linear), in units of turns (rows pre-scaled 0.5/pi).
  - range-reduce: d = t - RN(t) via the +2^23 round trick, sin via ACT Sin(2*pi*d)
  - envelopes via interp matmul (U), product on DVE, harmonic sum via PE matmul
    with 0/1 selection weights producing a [z*16+e, j] frame-major layout.
  - noise: windowed rDFT as matmuls (Hann folded into DFT matrix), per-frame Gaussian
    filter in freq domain, inverse rDFT as matmuls, overlap-add, PE transposes into
    the same frame-major layout.
  - final mix combine + strided DMA out.
"""
import os
import numpy as np
import ml_dtypes

B = 8
NE = 16
NH = 32
SEQ = 64
N = 16384
WS = 512
STEP = 256
NYQ = 11025.0
MIN_F0 = np.float32(20.0 / NYQ)
MAX_F0 = np.float32(800.0 / NYQ)
F0_DIFF = np.float32(MAX_F0 - MIN_F0)
NROW = NE * 33          # 528 osc rows (fundamental + 32 harmonics per event)
NBLK = 5                # 640 padded rows / 128
C23 = float(2.0 ** 23)
NFR = SEQ * NE          # 1024 frames per core (s-major: frame = s*16 + e)

_cache = {}


def _build_consts():
    if "consts" in _cache:
        return _cache["consts"]
    # linear-interp matrix U[k, i] (torch F.interpolate linear, align_corners=False)
    pos = (np.arange(N, dtype=np.float64) + 0.5) * (SEQ / N) - 0.5
    pos = np.clip(pos, 0.0, SEQ - 1)
    i0 = np.floor(pos).astype(np.int64)
    i1 = np.minimum(i0 + 1, SEQ - 1)
    w = pos - i0
    U = np.zeros((SEQ, N), dtype=np.float64)
    U[i0, np.arange(N)] += 1.0 - w
    U[i1, np.arange(N)] += w
    V = np.cumsum(U, axis=1)
    vh = V.astype(np.float16)
    vl = (V - vh.astype(np.float64)).astype(np.float16)
    V64 = np.concatenate([vh, vl], axis=1)                                # (64, 2N) fp16
    U32 = U.astype(ml_dtypes.bfloat16)

    # DFT matrices; Hann window folded into the forward transform
    t = np.arange(WS)
    f = np.arange(WS // 2 + 1)
    win = 0.5 - 0.5 * np.cos(2.0 * np.pi * t / WS)
    ang = 2.0 * np.pi * np.outer(t, f) / WS
    CwRe = (np.cos(ang) * win[:, None]).astype(ml_dtypes.bfloat16)    # (512, 257)
    CwIm = (-np.sin(ang) * win[:, None]).astype(ml_dtypes.bfloat16)
    cwnyq = np.stack([CwRe[:, 256], CwIm[:, 256]], axis=1).copy()  # (512, 2)
    wgt = np.full(WS // 2 + 1, 2.0)
    wgt[0] = 1.0
    wgt[-1] = 1.0
    ang2 = 2.0 * np.pi * np.outer(f, t) / WS
    DReF = (wgt[:, None] * np.cos(ang2) / WS).astype(ml_dtypes.bfloat16)   # (257, 512)
    DImF = (-wgt[:, None] * np.sin(ang2) / WS).astype(ml_dtypes.bfloat16)
    DRe = DReF[0:256].copy()
    DIm = DImF[0:256].copy()
    dnyq = np.stack([DReF[256], DImF[256]], axis=0).copy()     # (2, 512)

    p = np.arange(128, dtype=np.float32)
    freqcol = np.stack([p / 256.0, (128 + p) / 256.0, np.ones(128, np.float32)], axis=1)

    ident = np.eye(128, dtype=np.float32)
    identb = np.eye(128, dtype=ml_dtypes.bfloat16)

    sel2 = np.zeros((128, NBLK, 32), dtype=np.float32)
    for g in range(NROW):
        blk, rr = divmod(g, 128)
        e = g // 33
        sel2[rr, blk, e] = 1.0
        sel2[rr, blk, 16 + e] = 1.0
    sel2 = sel2.reshape(128, NBLK * 32).astype(ml_dtypes.bfloat16)

    consts = dict(Vc=V64, Uc=U32, CwRe=CwRe, CwIm=CwIm, cwnyq=cwnyq,
                  DRe=DRe, DIm=DIm, dnyq=dnyq, freqcol=freqcol, ident=ident,
                  identb=identb, sel2=sel2)
    _cache["consts"] = consts
    return consts


def _build_nc():
    if "nc" in _cache:
        return _cache["nc"]
    import concourse.bass as bass
    from concourse import bacc
    import concourse.tile as tile
    from concourse import mybir
    from contextlib import ExitStack

    F32 = mybir.dt.float32
    BF16 = mybir.dt.bfloat16
    AF = mybir.ActivationFunctionType
    OP = mybir.AluOpType

    nc = bacc.Bacc()
    frT = nc.declare_dram_parameter("frT", [64, 1280], mybir.dt.float16, isOutput=False)
    envT = nc.declare_dram_parameter("envT", [64, 640], BF16, isOutput=False)
    ovT2 = nc.declare_dram_parameter("ovT2", [64, 32], BF16, isOutput=False)
    mcrow = nc.declare_dram_parameter("mcrow", [2, NFR], F32, isOutput=False)
    nf = nc.declare_dram_parameter("nf", [NFR, WS], F32, isOutput=False)
    Vc = nc.declare_dram_parameter("Vc", [64, 2 * N], mybir.dt.float16, isOutput=False)
    Uc = nc.declare_dram_parameter("Uc", [64, N], BF16, isOutput=False)
    CwRe = nc.declare_dram_parameter("CwRe", [WS, 257], BF16, isOutput=False)
    CwIm = nc.declare_dram_parameter("CwIm", [WS, 257], BF16, isOutput=False)
    cwnyq = nc.declare_dram_parameter("cwnyq", [WS, 2], BF16, isOutput=False)
    DRe = nc.declare_dram_parameter("DRe", [256, WS], BF16, isOutput=False)
    DIm = nc.declare_dram_parameter("DIm", [256, WS], BF16, isOutput=False)
    dnyq = nc.declare_dram_parameter("dnyq", [2, WS], BF16, isOutput=False)
    freqcol = nc.declare_dram_parameter("freqcol", [128, 3], F32, isOutput=False)
    ident = nc.declare_dram_parameter("ident", [128, 128], F32, isOutput=False)
    identb = nc.declare_dram_parameter("identb", [128, 128], BF16, isOutput=False)
    sel2 = nc.declare_dram_parameter("sel2", [128, NBLK * 32], BF16, isOutput=False)
    out = nc.declare_dram_parameter("out", [NE, N], F32, isOutput=True)

    with tile.TileContext(nc) as tc, ExitStack() as ctx:
        cp = ctx.enter_context(tc.tile_pool(name="const", bufs=1))
        frT_sb = cp.tile([64, 1280], mybir.dt.float16, tag="frT")
        nc.sync.dma_start(frT_sb[:], frT[:])
        envT_sb = cp.tile([64, 640], BF16, tag="envT")
        nc.sync.dma_start(envT_sb[:], envT[:])
        ovT2_sb = cp.tile([64, 32], BF16, tag="ovT2")
        nc.sync.dma_start(ovT2_sb[:], ovT2[:])
        sel2_sb = cp.tile([128, NBLK * 32], BF16, tag="sel2")
        nc.sync.dma_start(sel2_sb[:], sel2[:])
        ident_sb = cp.tile([128, 128], F32, tag="ident")
        nc.sync.dma_start(ident_sb[:], ident[:])
        identb_sb = cp.tile([128, 128], BF16, tag="identb")
        nc.sync.dma_start(identb_sb[:], identb[:])
        b23 = cp.tile([128, 1], F32, tag="b23")
        nc.vector.memset(b23[:], C23)
        freqcol_sb = cp.tile([128, 3], F32, tag="freqcol")
        nc.sync.dma_start(freqcol_sb[:], freqcol[:])

        vup = ctx.enter_context(tc.tile_pool(name="vup", bufs=1))
        v_all = vup.tile([64, 2 * N], mybir.dt.float16, tag="v_all")
        # first half of V (hi+lo planes for chunks 0-3) on SWDGE, interleaved so
        # chunk 0 unblocks fastest; second half rides the sync queue later.
        for q in [0, 4, 1, 5]:
            nc.gpsimd.dma_start(v_all[:, q * (N // 4):(q + 1) * (N // 4)],
                                Vc[:, q * (N // 4):(q + 1) * (N // 4)])
        nzpool = ctx.enter_context(tc.tile_pool(name="nzT", bufs=1))
        nzT = [nzpool.tile([128, 256], BF16, tag=f"nzT{c}", name=f"nzT{c}") for c in range(8)]

        # osc-phase SBUF pools FIRST so their addresses don't overlap noise tiles
        vu = ctx.enter_context(tc.tile_pool(name="vu", bufs=2))
        ob = ctx.enter_context(tc.tile_pool(name="ob", bufs=4))
        oc = ctx.enter_context(tc.tile_pool(name="oc", bufs=1))
        # shared PSUM pools for BOTH phases (no bank-reuse wall): 4 + 1 + 3 = 8 banks
        psW = ctx.enter_context(tc.tile_pool(name="psW", bufs=2, space="PSUM"))
        psT2 = ctx.enter_context(tc.tile_pool(name="psT2", bufs=1, space="PSUM"))
        psO = ctx.enter_context(tc.tile_pool(name="psO", bufs=1, space="PSUM"))

        # ================= Phase A: noise =================
        if True:
            na = ctx.enter_context(tc.tile_pool(name="na", bufs=1))
            nf2 = ctx.enter_context(tc.tile_pool(name="nf2", bufs=2))
            mr = na.tile([1, NFR], F32, tag="mr")
            nc.sync.dma_start(mr[:], mcrow[0:1, :])
            c2r = na.tile([1, NFR], F32, tag="c2r")
            nc.sync.dma_start(c2r[:], mcrow[1:2, :])
            cwre_sb = cp.tile([128, 4 * 257], BF16, tag="cwre")
            cwim_sb = cp.tile([128, 4 * 257], BF16, tag="cwim")
            cwnyq_sb = cp.tile([128, 8], BF16, tag="cwnyq")
            dre_sb = cp.tile([128, 1024], BF16, tag="dre")
            dim_sb = cp.tile([128, 1024], BF16, tag="dim")
            dnyq_sb = cp.tile([2, WS], BF16, tag="dnyq")
            mean_bc = na.tile([128, NFR], F32, tag="meanbc")
            nc.gpsimd.partition_broadcast(mean_bc[:], mr[:])
            c2_bc = na.tile([128, NFR], F32, tag="c2bc")
            nc.gpsimd.partition_broadcast(c2_bc[:], c2r[:])

            # gaussian filters per freq chunk: exp(c2*(freq-mean)^2)
            filts = []
            for fc in range(2):
                fa = na.tile([128, NFR], F32, tag="fa")
                nc.vector.tensor_scalar(fa[:], mean_bc[:], freqcol_sb[:, fc:fc + 1], None, OP.subtract)
                fb = na.tile([128, NFR], F32, tag="fb")
                nc.scalar.activation(fb[:], fa[:], AF.Square)
                fm = na.tile([128, NFR], F32, tag="fm")
                nc.vector.tensor_tensor(fm[:], fb[:], c2_bc[:], OP.mult)
                ff = na.tile([128, NFR], BF16, tag=f"filt{fc}")
                nc.scalar.activation(ff[:], fm[:], AF.Exp)
                filts.append(ff)
            fan = na.tile([2, NFR], F32, tag="fa")
            nc.vector.tensor_scalar(fan[:], mean_bc[0:2, :], freqcol_sb[0:2, 2:3], None, OP.subtract)
            fbn = na.tile([2, NFR], F32, tag="fb")
            nc.scalar.activation(fbn[:], fan[:], AF.Square)
            fmn = na.tile([2, NFR], F32, tag="fm")
            nc.vector.tensor_tensor(fmn[:], fbn[:], c2_bc[0:2, :], OP.mult)
            filtn = na.tile([2, NFR], BF16, tag="filtn")
            nc.scalar.activation(filtn[:], fmn[:], AF.Exp)

            # transpose noise frames: nf [1024 fr, 512 t] -> xT[t4] [128 t, 1024 fr]
            xT = [na.tile([128, NFR], BF16, tag=f"xt{t4}", name=f"xt{t4}") for t4 in range(4)]
            for frb in range(8):
                nft = nf2.tile([128, WS], F32, tag="nf")
                nc.sync.dma_start(nft[:], nf[frb * 128:(frb + 1) * 128, :])
                for t4 in range(4):
                    ptr = psT2.tile([128, 128], F32, tag="tr")
                    nc.tensor.transpose(ptr[:], nft[:, t4 * 128:(t4 + 1) * 128], ident_sb[:])
                    nc.scalar.copy(xT[t4][:, frb * 128:(frb + 1) * 128], ptr[:])

            for t4 in range(4):
                nc.sync.dma_start(cwre_sb[:, t4 * 257:(t4 + 1) * 257], CwRe[t4 * 128:(t4 + 1) * 128, :])
                nc.sync.dma_start(cwim_sb[:, t4 * 257:(t4 + 1) * 257], CwIm[t4 * 128:(t4 + 1) * 128, :])
                nc.sync.dma_start(cwnyq_sb[:, t4 * 2:(t4 + 1) * 2], cwnyq[t4 * 128:(t4 + 1) * 128, :])
            for fc in range(2):
                nc.sync.dma_start(dre_sb[:, fc * 512:(fc + 1) * 512], DRe[fc * 128:(fc + 1) * 128, :])
                nc.sync.dma_start(dim_sb[:, fc * 512:(fc + 1) * 512], DIm[fc * 128:(fc + 1) * 128, :])
            nc.sync.dma_start(dnyq_sb[:], dnyq[:])
            for q in [2, 6, 3, 7]:
                nc.sync.dma_start(v_all[:, q * (N // 4):(q + 1) * (N // 4)],
                                  Vc[:, q * (N // 4):(q + 1) * (N // 4)])
            # rfft (windowed) + gaussian filter
            specf = {}
            for nameq, cw_sb, fc in [("re0", cwre_sb, 0), ("re1", cwre_sb, 1),
                                     ("im0", cwim_sb, 0), ("im1", cwim_sb, 1)]:
                sf = na.tile([128, NFR], BF16, tag=f"sf{nameq}", name=f"sf{nameq}")
                for h in range(2):
                    sl = slice(h * 512, (h + 1) * 512)
                    sp = psT2.tile([128, 512], F32, tag="tr")
                    for t4 in range(4):
                        nc.tensor.matmul(sp[:],
                                         cw_sb[:, t4 * 257 + fc * 128: t4 * 257 + fc * 128 + 128],
                                         xT[t4][:, sl],
                                         start=(t4 == 0), stop=(t4 == 3))
                    nc.vector.tensor_tensor(sf[:, sl], sp[:], filts[fc][:, sl], OP.mult)
                specf[nameq] = sf
            sfn = na.tile([2, NFR], BF16, tag="sfn")
            for h in range(2):
                sl = slice(h * 512, (h + 1) * 512)
                spn = psT2.tile([2, 512], F32, tag="tr")
                for t4 in range(4):
                    nc.tensor.matmul(spn[:], cwnyq_sb[:, t4 * 2:(t4 + 1) * 2],
                                     xT[t4][:, sl], start=(t4 == 0), stop=(t4 == 3))
                nc.vector.tensor_tensor(sfn[:, sl], spn[:], filtn[:, sl], OP.mult)

            # irfft
            ys = []
            for tau in range(4):
                yt = na.tile([128, NFR], BF16, tag=f"y{tau}", name=f"y{tau}")
                for h in range(2):
                    sl = slice(h * 512, (h + 1) * 512)
                    yp = psT2.tile([128, 512], F32, tag="tr")
                    nc.tensor.matmul(yp[:], dre_sb[:, 0 * 512 + tau * 128: 0 * 512 + tau * 128 + 128],
                                     specf["re0"][:, sl], start=True, stop=False)
                    nc.tensor.matmul(yp[:], dre_sb[:, 1 * 512 + tau * 128: 1 * 512 + tau * 128 + 128],
                                     specf["re1"][:, sl], start=False, stop=False)
                    nc.tensor.matmul(yp[:], dim_sb[:, 0 * 512 + tau * 128: 0 * 512 + tau * 128 + 128],
                                     specf["im0"][:, sl], start=False, stop=False)
                    nc.tensor.matmul(yp[:], dim_sb[:, 1 * 512 + tau * 128: 1 * 512 + tau * 128 + 128],
                                     specf["im1"][:, sl], start=False, stop=False)
                    nc.tensor.matmul(yp[:], dnyq_sb[:, tau * 128:(tau + 1) * 128],
                                     sfn[:, sl], start=False, stop=True)
                    nc.scalar.copy(yt[:, sl], yp[:])
                ys.append(yt)

            # overlap-add (hop 256; frame shift s-1 == column shift -16 in s-major order)
            nzs = []
            for jc in range(2):
                nzt = na.tile([128, NFR], BF16, tag=f"nz{jc}")
                nc.gpsimd.tensor_tensor(nzt[:, 16:NFR], ys[jc][:, 16:NFR],
                                        ys[jc + 2][:, 0:NFR - 16], OP.add)
                nc.gpsimd.tensor_copy(nzt[:, 0:16], ys[jc][:, 0:16])
                nzs.append(nzt)
            # transpose to frame-major nzT[c] [128 fr, 256 j]
            for c in range(8):
                for jc in range(2):
                    ptr = psT2.tile([128, 128], BF16, tag="tr")
                    nc.tensor.transpose(ptr[:], nzs[jc][:, c * 128:(c + 1) * 128], identb_sb[:])
                    nc.scalar.copy(nzT[c][:, jc * 128:(jc + 1) * 128], ptr[:])

        # ================= Phase B: oscillator bank =================
        if True:
            for c in range(8):
                u_sb = vu.tile([64, 2048], BF16, tag="u")
                nc.sync.dma_start(u_sb[:], Uc[:, c * 2048:(c + 1) * 2048])
                posc = psO.tile([128, 512], F32, tag="osc")
                pmix = psO.tile([128, 512], F32, tag="mix")
                q3 = psO.tile([64, 512], F32, tag="q3")
                posc3 = q3[0:32, :]
                pmix3 = q3[32:64, :]
                for zp in range(4):
                    mdst = pmix3 if zp == 3 else pmix[32 * zp:32 * (zp + 1), :]
                    nc.tensor.matmul(mdst, ovT2_sb[:],
                                     u_sb[:, zp * 512:(zp + 1) * 512], start=True, stop=True)
                for b in range(NBLK):
                    for ns in range(4):
                        pt = psW.tile([128, 512], F32, tag="t")
                        vh = v_all[:, c * 2048 + ns * 512: c * 2048 + (ns + 1) * 512]
                        vl = v_all[:, N + c * 2048 + ns * 512: N + c * 2048 + (ns + 1) * 512]
                        fh = frT_sb[:, b * 128:(b + 1) * 128]
                        fl = frT_sb[:, 640 + b * 128: 640 + (b + 1) * 128]
                        nc.tensor.matmul(pt[:], fh, vh, start=True, stop=False)
                        nc.tensor.matmul(pt[:], fl, vh, start=False, stop=False)
                        nc.tensor.matmul(pt[:], fh, vl, start=False, stop=True)
                        yt = ob.tile([128, 512], F32, tag="y")
                        nc.scalar.activation(yt[:], pt[:], AF.Identity, bias=b23[:])
                        kt = ob.tile([128, 512], F32, tag="k")
                        nc.gpsimd.tensor_scalar(kt[:], yt[:], -C23, None, OP.add)
                        dt_ = ob.tile([128, 512], F32, tag="d")
                        nc.vector.tensor_tensor(dt_[:], pt[:], kt[:], OP.subtract)
                        st = ob.tile([128, 512], BF16, tag="s")
                        nc.scalar.activation(st[:], dt_[:], AF.Sin, scale=float(2 * np.pi))
                        pe = psW.tile([128, 512], F32, tag="e")
                        nc.tensor.matmul(pe[:], envT_sb[:, b * 128:(b + 1) * 128],
                                         u_sb[:, ns * 512:(ns + 1) * 512], start=True, stop=True)
                        pr = ob.tile([128, 512], BF16, tag="p")
                        nc.vector.tensor_tensor(pr[:], st[:], pe[:], OP.mult)
                        odst = posc3 if ns == 3 else posc[32 * ns:32 * (ns + 1), :]
                        nc.tensor.matmul(odst, sel2_sb[:, b * 32:(b + 1) * 32],
                                         pr[:], start=(b == 0), stop=(b == NBLK - 1),
                                         skip_group_check=True)
                # final combine: out = mix*(osc - noise) + noise, split even/odd z halves
                a1 = oc.tile([128, 256], F32, tag="a1")
                nc.vector.tensor_tensor(a1[0:96, :], posc[0:96, 0:256], nzT[c][0:96, :], OP.subtract)
                nc.vector.tensor_tensor(a1[96:128, :], posc3[0:32, 0:256], nzT[c][96:128, :], OP.subtract)
                a2 = oc.tile([128, 256], F32, tag="a2")
                nc.vector.tensor_tensor(a2[0:96, :], posc[0:96, 256:512], nzT[c][0:96, :], OP.subtract)
                nc.vector.tensor_tensor(a2[96:128, :], posc3[0:32, 256:512], nzT[c][96:128, :], OP.subtract)
                b1 = oc.tile([128, 256], F32, tag="b1")
                nc.vector.tensor_tensor(b1[0:96, :], a1[0:96, :], pmix[0:96, 0:256], OP.mult)
                nc.vector.tensor_tensor(b1[96:128, :], a1[96:128, :], pmix3[0:32, 0:256], OP.mult)
                b2 = oc.tile([128, 256], F32, tag="b2")
                nc.vector.tensor_tensor(b2[0:96, :], a2[0:96, :], pmix[0:96, 256:512], OP.mult)
                nc.vector.tensor_tensor(b2[96:128, :], a2[96:128, :], pmix3[0:32, 256:512], OP.mult)
                c1 = oc.tile([128, 256], F32, tag="c1")
                nc.gpsimd.tensor_tensor(c1[:], b1[:], nzT[c][:], OP.add)
                c2t = oc.tile([128, 256], F32, tag="c2")
                nc.gpsimd.tensor_tensor(c2t[:], b2[:], nzT[c][:], OP.add)
                for z in range(8):
                    srct = c1 if z % 2 == 0 else c2t
                    nc.sync.dma_start(out[:, c * 2048 + z * 256: c * 2048 + (z + 1) * 256],
                                      srct[16 * z:16 * (z + 1), :])
    nc.finalize()
    _cache["nc"] = nc
    return nc


def kernel(**inputs):
    from concourse.bass_utils import run_bass_kernel_spmd

    f0 = np.asarray(inputs["f0"], np.float32)
    overall_env = np.asarray(inputs["overall_env"], np.float32)
    osc_env = np.asarray(inputs["osc_env"], np.float32)
    harm_env = np.asarray(inputs["harm_env"], np.float32)
    noise_std = np.asarray(inputs["noise_std"], np.float32)
    f0b = np.asarray(inputs["f0_baselines"], np.float32)
    noise_frames = np.asarray(inputs["noise_frames"], np.float32)

    # host prep (tiny, O(B*E*H*S))
    f0c = np.clip(f0, -0.5, 0.5)
    erb = ((0.108 * (f0b * NYQ) + 24.7) / NYQ).astype(np.float32)
    f0v = np.clip(f0b + f0c * erb, 0.0, 1.0).astype(np.float32)
    f0n = (MIN_F0 + f0v * F0_DIFF).astype(np.float32)                     # (8,16,64)
    hfact = np.concatenate([[1.0], np.arange(2, 2 + NH)]).astype(np.float32)
    freq_rows = f0n[:, :, None, :] * hfact[None, None, :, None] * np.float32(0.5)
    fr_t = np.zeros((B, 64, 640), np.float32)
    fr_t[:, :, 0:NROW] = freq_rows.reshape(B, NROW, SEQ).transpose(0, 2, 1)
    fh = fr_t.astype(np.float16)
    fl = (fr_t - fh.astype(np.float32)).astype(np.float16)
    frT = np.concatenate([fh, fl], axis=2)                                # (B,64,1280)
    oe = np.clip(osc_env, 0.0, 1.0).astype(np.float32)
    he = np.clip(harm_env, 0.0, 1.0).astype(np.float32)
    env_rows = oe[:, :, None, :] * np.concatenate(
        [np.ones((B, NE, 1, SEQ), np.float32), he], axis=2)
    envT = np.zeros((B, 64, 640), ml_dtypes.bfloat16)
    envT[:, :, 0:NROW] = env_rows.reshape(B, NROW, SEQ).transpose(0, 2, 1).astype(ml_dtypes.bfloat16)
    ov = np.clip(overall_env, 0.0, 1.0).astype(np.float32)
    ovT = ov.transpose(0, 2, 1)                                           # (8,64,16)
    ovT2 = np.concatenate([ovT, ovT], axis=2).astype(ml_dtypes.bfloat16)  # (8,64,32)
    std = (np.clip(noise_std, 1e-12, 1.0) * F0_DIFF).astype(np.float32)
    c2 = (-0.5 / (std.astype(np.float64) ** 2)).astype(np.float32)
    mcrow = np.stack([f0n.transpose(0, 2, 1).reshape(B, NFR),
                      c2.transpose(0, 2, 1).reshape(B, NFR)], axis=1).astype(np.float32)
    nf = np.ascontiguousarray(noise_frames.transpose(0, 2, 1, 3).reshape(B, NFR, WS))

    consts = _build_consts()
    nc = _build_nc()

    in_maps = []
    for b in range(B):
        m = dict(frT=frT[b], envT=envT[b], ovT2=ovT2[b], mcrow=mcrow[b], nf=nf[b])
        m.update(consts)
        in_maps.append(m)

    trace = bool(os.environ.get("BASS_PROFILE"))
    res = run_bass_kernel_spmd(nc, in_maps, list(range(B)), trace=trace)
    if trace and res.exec_time_ns is not None:
        print(f"HW exec time: {res.exec_time_ns} ns")
    out = np.stack([r["out"] for r in res.results]).astype(np.float32)
    return out



# revision 18
# speedup vs baseline: 1.4202x; 1.4202x over previous
"""Trainium2 Bass kernel for nn_AudioEvent: oscillator bank + FFT-filtered noise synth.

Sharding: data-parallel over batch (B=8) -> one batch element per NeuronCore.

v2 design (chunk-basis formulation):
  - phase(n) within each 512-sample chunk = frac(prefix) + 4-knot coef @ fixed
    cumulative basis -> ONE f32r matmul per (block, chunk) at 1 cyc/row.
  - range reduction: single DVE/Pool tensor_scalar mod(phase, 1.0) -> fp16.
  - sin via ACT with scale just under 2*pi (arg stays inside [-pi, pi]);
    the sign flip from biasing by -pi is folded into negated selection weights.
  - envelope interp = 4-knot coef @ per-sample basis (f32r matmul).
  - harmonic sum via two 256-wide selection matmuls per (block, chunk) that
    land posc directly in z-major [16z+e, 256j] layout; mix interp lands in
    the other half of the same PSUM bank.
  - noise: host pre-transposed windowed frames; rDFT as fp16 matmuls keeping
    only bins 0..127 (gaussian filter ~0 above 0.5 Nyquist for this f0/std
    range); inverse rDFT fused with overlap-add via frame-shifted stationary
    operand, output directly in z-major layout.
"""
import os
import numpy as np

B = 8
NE = 16
NH = 32
SEQ = 64
N = 16384
WS = 512
NYQ = 11025.0
MIN_F0 = np.float32(20.0 / NYQ)
MAX_F0 = np.float32(800.0 / NYQ)
F0_DIFF = np.float32(MAX_F0 - MIN_F0)
NROW = NE * 33          # 528 osc rows
NBLK = 5                # 640 padded rows / 128
NFR = SEQ * NE          # 1024 frames per core (frame = s*16 + e)
CH = 512                # chunk width
NCH = N // CH           # 32 chunks
NGRP = 8                # groups of 4 chunks (2048 samples)
NBINS = 128             # kept rfft bins

# largest fp32 strictly below 2*pi: mod output [0,1] maps inside [-pi, pi]
SIN_SCALE = float(np.float32(6.28318500518798828125))
SIN_BIAS = -SIN_SCALE / 2.0

_cache = {}


def _interp_u():
    pos = (np.arange(N, dtype=np.float64) + 0.5) * (SEQ / N) - 0.5
    pos = np.clip(pos, 0.0, SEQ - 1)
    i0 = np.floor(pos).astype(np.int64)
    i1 = np.minimum(i0 + 1, SEQ - 1)
    w = pos - i0
    U = np.zeros((SEQ, N))
    U[i0, np.arange(N)] += 1.0 - w
    U[i1, np.arange(N)] += w
    return U


def _build_consts():
    if "consts" in _cache:
        return _cache["consts"]
    U = _interp_u()

    # chunk-local bases from an interior chunk (exact everywhere with the
    # extended-knot convention fx[k] = f[clip(k-1, 0, 63)])
    c0 = 4
    Bp = np.zeros((5, CH))
    Benv = np.zeros((4, CH))
    Bp[0] = 1.0
    for j in range(4):
        u_j = U[2 * c0 - 1 + j, CH * c0: CH * (c0 + 1)]
        Benv[j] = u_j
        Bp[1 + j] = np.cumsum(u_j)
    bh = Bp.astype(np.float16)
    bl = (Bp - bh.astype(np.float64)).astype(np.float16)
    Bp = np.ascontiguousarray(np.concatenate([bh, bh, bl], axis=0))  # (15, 512)
    Benv = np.ascontiguousarray(Benv.astype(np.float16))

    # forward rDFT (t-major), bins 0..127 only; Hann window is pre-applied to
    # the noise frames on the host
    t = np.arange(WS)
    f = np.arange(NBINS)
    ang = 2.0 * np.pi * np.outer(t, f) / WS
    CwRe = np.ascontiguousarray(np.cos(ang).astype(np.float16))
    CwIm = np.ascontiguousarray((-np.sin(ang)).astype(np.float16))

    # inverse rDFT bases (OLA-fused: cols 0:256 -> y[j], cols 256:512 -> y[j+256])
    wgt = np.full(NBINS, 2.0)
    wgt[0] = 1.0
    ang2 = 2.0 * np.pi * np.outer(f, t) / WS
    Dre = np.ascontiguousarray((wgt[:, None] * np.cos(ang2) / WS).astype(np.float16))
    Dim = np.ascontiguousarray((-wgt[:, None] * np.sin(ang2) / WS).astype(np.float16))

    freqcol = (np.arange(NBINS, dtype=np.float32) / 256.0).reshape(NBINS, 1).copy()

    # within-segment interp weight profile for a 256-sample z-slice:
    # j < 128 -> (j+128.5)/256 (right knot weight), j >= 128 -> (j-127.5)/256
    j = np.arange(256)
    w1 = np.where(j < 128, (j + 128.5) / 256.0, (j - 127.5) / 256.0)
    w1bc = np.ascontiguousarray(
        np.broadcast_to(w1.astype(np.float16), (128, 256)).copy())
    w0bc = np.ascontiguousarray(
        np.broadcast_to((1.0 - w1).astype(np.float16), (128, 256)).copy())

    consts = dict(Bp=Bp, Benv=Benv, CwRe=CwRe, CwIm=CwIm,
                  Dre=Dre, Dim=Dim, freqcol=freqcol, w1bc=w1bc, w0bc=w0bc)
    _cache["consts"] = consts
    return consts


def _host_prep(inputs):
    """Vectorized host prep over all cores. Returns per-core input maps."""
    f0 = np.asarray(inputs["f0"], np.float32)
    overall_env = np.asarray(inputs["overall_env"], np.float32)
    osc_env = np.asarray(inputs["osc_env"], np.float32)
    harm_env = np.asarray(inputs["harm_env"], np.float32)
    noise_std = np.asarray(inputs["noise_std"], np.float32)
    f0b = np.asarray(inputs["f0_baselines"], np.float32)
    noise_frames = np.asarray(inputs["noise_frames"], np.float32)

    U = _interp_u()

    # match the reference's fp32 frequency values exactly: f0n and f0n*h are
    # rounded to fp32 there, and the phase trajectory follows those values
    f0c = np.clip(f0, -0.5, 0.5)
    erb = ((0.108 * (f0b * NYQ) + 24.7) / NYQ).astype(np.float32)
    f0v = np.clip(f0b + f0c * erb, 0.0, 1.0).astype(np.float32)
    f0n32 = (MIN_F0 + f0v * F0_DIFF).astype(np.float32)          # (B,16,64)
    hfact32 = np.arange(2, 2 + NH, dtype=np.float32)
    harm32 = (f0n32[:, :, None, :] * hfact32[None, None, :, None]).astype(np.float32)
    rows32 = np.concatenate([f0n32[:, :, None, :], harm32], axis=2)  # (B,16,33,64)
    fT = rows32.astype(np.float64).reshape(B, NROW, SEQ) * 0.5   # turns/sample
    f0n = f0n32.astype(np.float64)

    # chunk prefixes P[g, c] = phase before chunk c (fp64 exact), frac-reduced
    V = np.cumsum(U, axis=1)
    Vc = V[:, [CH * c - 1 for c in range(1, NCH)]]               # (64, 31)
    P = np.zeros((B, NROW, NCH))
    P[:, :, 1:] = fT @ Vc

    # extended knots fx[:, :, kk] = f[:, :, clip(kk-1, 0, 63)], kk = 0..65
    ext = np.clip(np.arange(SEQ + 2) - 1, 0, SEQ - 1)

    fx_pad = np.zeros((B, NBLK * 128, SEQ + 2), np.float64)
    fx_pad[:, :NROW] = fT[:, :, ext]
    Pf_pad = np.zeros((B, NBLK * 128, NCH), np.float64)
    Pf_pad[:, :NROW] = np.mod(P, 1.0)

    # phase coefs frc[b][15, NCH*NBLK*128]: hi/lo fp16 split [ch; cl; ch],
    # free index = (c*NBLK + blk)*128 + r
    frc = np.zeros((B, NCH, NBLK, 5, 128), np.float64)
    for c in range(NCH):
        blkv = fx_pad[:, :, 2 * c: 2 * c + 4]                    # (B,640,4)
        kn = blkv.reshape(B, NBLK, 128, 4).transpose(0, 1, 3, 2)  # (B,5,4,128)
        frc[:, c, :, 1:5, :] = kn
        frc[:, c, :, 0, :] = Pf_pad[:, :, c].reshape(B, NBLK, 128)
    ch = frc.astype(np.float16)
    cl = (frc - ch.astype(np.float64)).astype(np.float16)
    frc = np.concatenate([ch, cl, ch], axis=3)                   # (B,NCH,NBLK,15,128)
    frc = np.ascontiguousarray(
        frc.transpose(0, 3, 1, 2, 4).reshape(B, 15, NCH * NBLK * 128))

    # env-valued (negated) selection weights per extended knot kk and block b:
    # layout [128, 16 + (kk*NBLK+b)*32 ...]: 16 leading zeros, then per index
    # i = kk*NBLK+b a [16 w-cols | 16 zero-cols] pair. The u=0 stationary
    # slice is cols [16+32i, 16+32i+32) = [w | 0]; u=1 is [32i, 32i+32) = [0 | w].
    oe = np.clip(osc_env, 0.0, 1.0).astype(np.float32)
    he = np.clip(harm_env, 0.0, 1.0).astype(np.float32)
    env_rows = oe[:, :, None, :] * np.concatenate(
        [np.ones((B, NE, 1, SEQ), np.float32), he], axis=2)
    env_rows = env_rows.reshape(B, NROW, SEQ)
    NKK = SEQ + 2
    evw = np.zeros((B, NKK, NBLK, 128, 16), np.float16)
    gidx = np.arange(NROW)
    ev = gidx // 33
    blk = gidx // 128
    rr = gidx % 128
    envx = -env_rows[:, :, ext]                                  # (B,528,66)
    evw[:, :, blk, rr, ev] = envx[:, gidx].transpose(0, 2, 1)
    ewpad = np.zeros((B, NKK, NBLK, 128, 32), np.float16)
    ewpad[..., 0:16] = evw
    ew = np.zeros((B, 128, 16 + NKK * NBLK * 32), np.float16)
    ew[:, :, 16:] = ewpad.transpose(0, 3, 1, 2, 4).reshape(B, 128, NKK * NBLK * 32)
    ew = np.ascontiguousarray(ew)

    # overall mix coefs, u-split: ovc[b][4, (c*2+u)*32 + row], zero outside
    # the u-th 16-row group
    ov = np.clip(overall_env, 0.0, 1.0).astype(np.float32)
    ovx = ov[:, :, ext]                                          # (B,16,66)
    ovc = np.zeros((B, NCH, 2, 4, 32), np.float16)
    for c in range(NCH):
        k = ovx[:, :, 2 * c: 2 * c + 4].transpose(0, 2, 1)       # (B,4,16)
        ovc[:, c, 0, :, 0:16] = k
        ovc[:, c, 1, :, 16:32] = k
    ovc = np.ascontiguousarray(
        ovc.transpose(0, 3, 1, 2, 4).reshape(B, 4, NCH * 64))

    # noise: mean/c2 rows frame-major (fr = s*16 + e)
    std = (np.clip(noise_std, 1e-12, 1.0) * F0_DIFF).astype(np.float32)
    c2 = (-0.5 / (std.astype(np.float64) ** 2)).astype(np.float32)
    f0n32 = f0n.astype(np.float32)
    mcrow = np.ascontiguousarray(
        np.stack([f0n32.transpose(0, 2, 1).reshape(B, NFR),
                  c2.transpose(0, 2, 1).reshape(B, NFR)], axis=1))

    # pre-windowed, transposed noise frames: nfT[b][t, fr] fp16
    t = np.arange(WS)
    win = (0.5 - 0.5 * np.cos(2.0 * np.pi * t / WS)).astype(np.float32)
    nf = noise_frames.transpose(0, 2, 1, 3).reshape(B, NFR, WS)  # (B, fr, t)
    nfT = np.ascontiguousarray(
        (nf * win[None, None, :]).transpose(0, 2, 1).astype(np.float16))

    consts = _build_consts()
    in_maps = []
    for b in range(B):
        m = dict(frc=frc[b], ew=ew[b], ovc=ovc[b], mcrow=mcrow[b], nfT=nfT[b])
        m.update(consts)
        in_maps.append(m)
    return in_maps


def _build_nc():
    if "nc" in _cache:
        return _cache["nc"]
    from concourse import bacc
    import concourse.tile as tile
    from concourse import mybir
    from contextlib import ExitStack

    F32 = mybir.dt.float32
    F32R = mybir.dt.float32r
    FP16 = mybir.dt.float16
    AF = mybir.ActivationFunctionType
    OP = mybir.AluOpType

    nc = bacc.Bacc()
    frc = nc.declare_dram_parameter("frc", [15, NCH * NBLK * 128], FP16, isOutput=False)
    ew = nc.declare_dram_parameter("ew", [128, 16 + (SEQ + 2) * NBLK * 32], FP16, isOutput=False)
    ovc = nc.declare_dram_parameter("ovc", [4, NCH * 64], FP16, isOutput=False)
    mcrow = nc.declare_dram_parameter("mcrow", [2, NFR], F32, isOutput=False)
    nfT = nc.declare_dram_parameter("nfT", [WS, NFR], FP16, isOutput=False)
    Bp = nc.declare_dram_parameter("Bp", [15, CH], FP16, isOutput=False)
    Benv = nc.declare_dram_parameter("Benv", [4, CH], FP16, isOutput=False)
    CwRe = nc.declare_dram_parameter("CwRe", [WS, NBINS], FP16, isOutput=False)
    CwIm = nc.declare_dram_parameter("CwIm", [WS, NBINS], FP16, isOutput=False)
    Dre = nc.declare_dram_parameter("Dre", [NBINS, WS], FP16, isOutput=False)
    Dim = nc.declare_dram_parameter("Dim", [NBINS, WS], FP16, isOutput=False)
    freqcol = nc.declare_dram_parameter("freqcol", [NBINS, 1], F32, isOutput=False)
    w1bc = nc.declare_dram_parameter("w1bc", [128, 256], FP16, isOutput=False)
    w0bc = nc.declare_dram_parameter("w0bc", [128, 256], FP16, isOutput=False)
    out = nc.declare_dram_parameter("out", [NE, N], F32, isOutput=True)

    CB = 2 * NBLK * 128   # coef columns per c2 iteration (1280)

    with tile.TileContext(nc) as tc, ExitStack() as ctx:
        cp = ctx.enter_context(tc.tile_pool(name="const", bufs=1))

        # noise inputs first so the noise pipeline starts ASAP
        mr_sb = cp.tile([1, NFR], F32, tag="mr")
        nc.sync.dma_start(mr_sb[:], mcrow[0:1, :])
        c2r_sb = cp.tile([1, NFR], F32, tag="c2r")
        nc.sync.dma_start(c2r_sb[:], mcrow[1:2, :])
        cwre_sb = cp.tile([128, 4 * NBINS], FP16, tag="cwre")
        cwim_sb = cp.tile([128, 4 * NBINS], FP16, tag="cwim")
        for t4 in range(4):
            nc.sync.dma_start(cwre_sb[:, t4 * NBINS:(t4 + 1) * NBINS],
                              CwRe[t4 * 128:(t4 + 1) * 128, :])
            nc.sync.dma_start(cwim_sb[:, t4 * NBINS:(t4 + 1) * NBINS],
                              CwIm[t4 * 128:(t4 + 1) * 128, :])
        nft_sb = [cp.tile([128, NFR], FP16, tag=f"nft{t4}", name=f"nft{t4}")
                  for t4 in range(4)]
        for t4 in range(4):
            nc.sync.dma_start(nft_sb[t4][:], nfT[t4 * 128:(t4 + 1) * 128, :])
        freqcol_sb = cp.tile([NBINS, 1], F32, tag="freqcol")
        nc.sync.dma_start(freqcol_sb[:], freqcol[:])
        dre_sb = cp.tile([NBINS, WS], FP16, tag="dre")
        nc.sync.dma_start(dre_sb[:], Dre[:])
        dim_sb = cp.tile([NBINS, WS], FP16, tag="dim")
        nc.sync.dma_start(dim_sb[:], Dim[:])

        # osc constants
        bp_sb = cp.tile([15, CH], FP16, tag="bp")
        nc.sync.dma_start(bp_sb[:], Bp[:])
        benv_sb = cp.tile([4, CH], FP16, tag="benv")
        nc.sync.dma_start(benv_sb[:], Benv[:])
        w1_sb = cp.tile([128, 256], FP16, tag="w1bc")
        nc.sync.dma_start(w1_sb[:], w1bc[:])
        w0_sb = cp.tile([128, 256], FP16, tag="w0bc")
        nc.sync.dma_start(w0_sb[:], w0bc[:])
        ew_sb = cp.tile([128, 16 + (SEQ + 2) * NBLK * 32], FP16, tag="ew")
        nc.sync.dma_start(ew_sb[:], ew[:])
        ovc_sb = cp.tile([4, NCH * 64], FP16, tag="ovc")
        nc.sync.dma_start(ovc_sb[:], ovc[:])
        bsin = cp.tile([128, 1], F32, tag="bsin")
        nc.vector.memset(bsin[:], 0.0)
        b1024 = cp.tile([128, 1], F32, tag="b1024")
        nc.vector.memset(b1024[:], 1024.0)

        # ---------------- noise phase (PSUM pools scoped) ----------------
        na = ctx.enter_context(tc.tile_pool(name="na", bufs=1))
        nzS = [na.tile([128, 256], FP16, tag=f"nz{g}", name=f"nz{g}")
               for g in range(NGRP)]
        with tc.tile_pool(name="psN", bufs=2, space="PSUM") as psN, \
             tc.tile_pool(name="psZ", bufs=2, space="PSUM") as psZ:
            mean_bc = na.tile([128, NFR], F32, tag="meanbc")
            nc.gpsimd.partition_broadcast(mean_bc[:], mr_sb[:])
            c2_bc = na.tile([128, NFR], F32, tag="c2bc")
            nc.gpsimd.partition_broadcast(c2_bc[:], c2r_sb[:])
            # filt = exp(c2 * (freq - mean)^2)
            fb = na.tile([128, NFR], F32, tag="fb")
            nc.scalar.activation(fb[:], mean_bc[:], AF.Square,
                                 bias=freqcol_sb[:], scale=-1.0)
            fm = na.tile([128, NFR], F32, tag="fm")
            nc.gpsimd.tensor_tensor(fm[:], fb[:], c2_bc[:], OP.mult)
            ff = na.tile([128, NFR], FP16, tag="ff")
            nc.scalar.activation(ff[:], fm[:], AF.Exp)

            # rfft (bins 0..127) + filter; specf tiles have 16 zero lead cols
            sfre = na.tile([128, 16 + NFR], FP16, tag="sfre")
            sfim = na.tile([128, 16 + NFR], FP16, tag="sfim")
            nc.vector.memset(sfre[:, 0:16], 0.0)
            nc.vector.memset(sfim[:, 0:16], 0.0)
            for h in range(2):
                sl = slice(h * 512, (h + 1) * 512)
                spr = psN.tile([128, 512], F32, tag="spr")
                spi = psN.tile([128, 512], F32, tag="spi")
                for t4 in range(4):
                    nc.tensor.matmul(spr[:], cwre_sb[:, t4 * NBINS:(t4 + 1) * NBINS],
                                     nft_sb[t4][:, sl], start=(t4 == 0), stop=(t4 == 3))
                for t4 in range(4):
                    nc.tensor.matmul(spi[:], cwim_sb[:, t4 * NBINS:(t4 + 1) * NBINS],
                                     nft_sb[t4][:, sl], start=(t4 == 0), stop=(t4 == 3))
                nc.vector.tensor_tensor(sfre[:, 16 + h * 512:16 + (h + 1) * 512],
                                        spr[:], ff[:, sl], OP.mult)
                nc.vector.tensor_tensor(sfim[:, 16 + h * 512:16 + (h + 1) * 512],
                                        spi[:], ff[:, sl], OP.mult)

            # inverse rDFT + OLA fused; nzS[g] in z-major [16z+e, 256j] fp16
            for g in range(NGRP):
                nzp = psZ.tile([128, 256], F32, tag="nzp")
                nc.tensor.matmul(nzp[:], sfre[:, 16 + g * 128: 16 + g * 128 + 128],
                                 dre_sb[:, 0:256], start=True, stop=False)
                nc.tensor.matmul(nzp[:], sfim[:, 16 + g * 128: 16 + g * 128 + 128],
                                 dim_sb[:, 0:256], start=False, stop=False)
                nc.tensor.matmul(nzp[:], sfre[:, g * 128: g * 128 + 128],
                                 dre_sb[:, 256:512], start=False, stop=False)
                nc.tensor.matmul(nzp[:], sfim[:, g * 128: g * 128 + 128],
                                 dim_sb[:, 256:512], start=False, stop=True)
                nc.scalar.copy(nzS[g][:], nzp[:])

        # ---------------- oscillator phase ----------------
        cof = ctx.enter_context(tc.tile_pool(name="cof", bufs=3))
        dsp = ctx.enter_context(tc.tile_pool(name="dsp", bufs=8))
        stp = ctx.enter_context(tc.tile_pool(name="stp", bufs=7))
        ocp = ctx.enter_context(tc.tile_pool(name="ocp", bufs=2))
        psA = ctx.enter_context(tc.tile_pool(name="psA", bufs=2, space="PSUM"))
        psG = ctx.enter_context(tc.tile_pool(name="psG", bufs=2, space="PSUM"))
        psM = ctx.enter_context(tc.tile_pool(name="psM", bufs=2, space="PSUM"))

        def ew_sl(kk, b, u):
            i = kk * NBLK + b
            base = 16 + 32 * i if u == 0 else 32 * i
            return ew_sb[:, base: base + 32]

        gbans = {}
        pmbans = {}
        for c2 in range(NCH // 2):
            cA = 2 * c2
            grp = cA // 4
            if cA % 4 == 0:
                gbans[grp] = psG.tile([128, 512], F32, tag="gb", name=f"gb{grp}")
                pmbans[grp] = psM.tile([128, 256], F32, tag="pm", name=f"pm{grp}")
            gb = gbans[grp]
            pm = pmbans[grp]

            # stream this iteration's phase coefficients
            fc_sb = cof.tile([15, CB], FP16, tag="fc")
            nc.sync.dma_start(fc_sb[:], frc[:, c2 * CB:(c2 + 1) * CB])

            sts = []
            for b in range(NBLK):
                pa = psA.tile([128, 1024], F32, tag="pa")
                for ci in range(2):
                    idx = (ci * NBLK + b) * 128
                    nc.tensor.matmul(pa[:, ci * 512:(ci + 1) * 512],
                                     fc_sb[:, idx:idx + 128],
                                     bp_sb[:], start=True, stop=True)
                # range reduction: yt = fp16(phase + 1024) rounds to the
                # nearest integer (fp16 ulp is exactly 1 in [1024, 2048));
                # then -dt = (yt - 1024) - phase via scalar_tensor_tensor.
                yt = dsp.tile([128, 1024], FP16, tag="yt")
                if b < 3:
                    nc.scalar.activation(yt[:], pa[:], AF.Identity,
                                         bias=b1024[:], scale=1.0)
                else:
                    nc.vector.tensor_scalar(yt[:], pa[:], 1024.0, None, OP.add)
                dt_ = dsp.tile([128, 1024], FP16, tag="dt")
                nc.vector.scalar_tensor_tensor(dt_[:], yt[:], 1024.0, pa[:],
                                               OP.subtract, OP.subtract)
                st = stp.tile([128, 1024], FP16, tag="st")
                nc.scalar.activation(st[:], dt_[:], AF.Sin, bias=bsin[:],
                                     scale=SIN_SCALE)
                sts.append(st)

            # env-weighted selection matmuls: Glo (gb cols 0:256) holds the
            # left-knot weighted harmonic sum, Ghi (cols 256:512) the right
            for ci in range(2):
                cc = cA + ci
                zp = cc % 4
                for u in range(2):
                    mkk = 2 * cc + u
                    for b in range(NBLK):
                        stA = sts[b][:, ci * 512 + u * 256: ci * 512 + u * 256 + 128]
                        stB = sts[b][:, ci * 512 + u * 256 + 128: ci * 512 + (u + 1) * 256]
                        # start=True only on the very first matmul touching this
                        # 32-row bank region: its start marks the whole zero
                        # region pending, and first-touch zeroing initializes
                        # the other column ranges
                        fb_ = (u == 0 and b == 0)
                        lb_ = (u == 1 and b == NBLK - 1)
                        nc.tensor.matmul(gb[32 * zp:32 * zp + 32, 0:128],
                                         ew_sl(mkk, b, u), stA,
                                         start=fb_, stop=False,
                                         skip_group_check=True, tile_position=(0, 32 * zp))
                        nc.tensor.matmul(gb[32 * zp:32 * zp + 32, 128:256],
                                         ew_sl(mkk + 1, b, u), stB,
                                         start=False, stop=False,
                                         skip_group_check=True, tile_position=(0, 32 * zp))
                        nc.tensor.matmul(gb[32 * zp:32 * zp + 32, 256:384],
                                         ew_sl(mkk + 1, b, u), stA,
                                         start=False, stop=False,
                                         skip_group_check=True, tile_position=(0, 32 * zp))
                        nc.tensor.matmul(gb[32 * zp:32 * zp + 32, 384:512],
                                         ew_sl(mkk + 2, b, u), stB,
                                         start=False, stop=lb_,
                                         skip_group_check=True, tile_position=(0, 32 * zp))
                # mix interp into pm (z-major rows)
                for u in range(2):
                    nc.tensor.matmul(
                        pm[32 * zp:32 * zp + 32, 0:256],
                        ovc_sb[:, (cc * 2 + u) * 32:(cc * 2 + u + 1) * 32],
                        benv_sb[:, u * 256:(u + 1) * 256],
                        start=(u == 0), stop=(u == 1), skip_group_check=True,
                        tile_position=(0, 32 * zp))

            if cA % 4 == 2:
                # blend: posc = w0*Glo + w1*Ghi (each op reads <=1 PSUM input)
                glo = gb[:, 0:256]
                ghi = gb[:, 256:512]
                t1 = ocp.tile([128, 256], FP16, tag="t1")
                nc.vector.tensor_tensor(t1[:], glo, w0_sb[:], OP.mult)
                t2 = ocp.tile([128, 256], FP16, tag="t2")
                nc.vector.tensor_tensor(t2[:], ghi, w1_sb[:], OP.mult)
                posc_s = ocp.tile([128, 256], F32, tag="ps")
                nc.vector.tensor_tensor(posc_s[:], t1[:], t2[:], OP.add)
                pc = ocp.tile([128, 256], FP16, tag="pc")
                nc.scalar.copy(pc[:], pm[:])
                av = ocp.tile([128, 256], F32, tag="av")
                nc.gpsimd.tensor_tensor(av[:], posc_s[:], nzS[grp][:], OP.subtract)
                bv = ocp.tile([128, 256], F32, tag="bv")
                nc.gpsimd.tensor_tensor(bv[:], av[:], pc[:], OP.mult)
                ov_ = ocp.tile([128, 256], F32, tag="ov")
                nc.gpsimd.tensor_tensor(ov_[:], bv[:], nzS[grp][:], OP.add)
                for z in range(8):
                    nc.sync.dma_start(
                        out[:, grp * 2048 + z * 256: grp * 2048 + (z + 1) * 256],
                        ov_[16 * z:16 * (z + 1), :])
                del gbans[grp]
                del pmbans[grp]

    nc.finalize()
    _cache["nc"] = nc
    return nc


def kernel(**inputs):
    from concourse.bass_utils import run_bass_kernel_spmd

    in_maps = _host_prep(inputs)
    nc = _build_nc()

    trace = bool(os.environ.get("BASS_PROFILE"))
    res = run_bass_kernel_spmd(nc, in_maps, list(range(B)), trace=trace)
    if trace and res.exec_time_ns is not None:
        print(f"HW exec time: {res.exec_time_ns} ns")
    out = np.stack([r["out"] for r in res.results]).astype(np.float32)
    return out


# revision 25
# speedup vs baseline: 1.5687x; 1.1046x over previous
"""Trainium2 Bass kernel for nn_AudioEvent: oscillator bank + FFT-filtered noise synth.

Sharding: data-parallel over batch (B=8) -> one batch element per NeuronCore.

v2 design (chunk-basis formulation):
  - phase(n) within each 512-sample chunk = frac(prefix) + 4-knot coef @ fixed
    cumulative basis -> ONE f32r matmul per (block, chunk) at 1 cyc/row.
  - range reduction: single DVE/Pool tensor_scalar mod(phase, 1.0) -> fp16.
  - sin via ACT with scale just under 2*pi (arg stays inside [-pi, pi]);
    the sign flip from biasing by -pi is folded into negated selection weights.
  - envelope interp = 4-knot coef @ per-sample basis (f32r matmul).
  - harmonic sum via two 256-wide selection matmuls per (block, chunk) that
    land posc directly in z-major [16z+e, 256j] layout; mix interp lands in
    the other half of the same PSUM bank.
  - noise: host pre-transposed windowed frames; rDFT as fp16 matmuls keeping
    only bins 0..127 (gaussian filter ~0 above 0.5 Nyquist for this f0/std
    range); inverse rDFT fused with overlap-add via frame-shifted stationary
    operand, output directly in z-major layout.
"""
import os
import numpy as np

B = 8
NE = 16
NH = 32
SEQ = 64
N = 16384
WS = 512
NYQ = 11025.0
MIN_F0 = np.float32(20.0 / NYQ)
MAX_F0 = np.float32(800.0 / NYQ)
F0_DIFF = np.float32(MAX_F0 - MIN_F0)
NROW = NE * 33          # 528 osc rows
NBLK = 5                # 640 padded rows / 128
NFR = SEQ * NE          # 1024 frames per core (frame = s*16 + e)
CH = 512                # chunk width
NCH = N // CH           # 32 chunks
NGRP = 8                # groups of 4 chunks (2048 samples)
NBINS = 128             # kept rfft bins

# largest fp32 strictly below 2*pi: mod output [0,1] maps inside [-pi, pi]
SIN_SCALE = float(np.float32(6.28318500518798828125))
SIN_BIAS = -SIN_SCALE / 2.0

_cache = {}


def _interp_u():
    pos = (np.arange(N, dtype=np.float64) + 0.5) * (SEQ / N) - 0.5
    pos = np.clip(pos, 0.0, SEQ - 1)
    i0 = np.floor(pos).astype(np.int64)
    i1 = np.minimum(i0 + 1, SEQ - 1)
    w = pos - i0
    U = np.zeros((SEQ, N))
    U[i0, np.arange(N)] += 1.0 - w
    U[i1, np.arange(N)] += w
    return U


def _build_consts():
    if "consts" in _cache:
        return _cache["consts"]
    U = _interp_u()

    # chunk-local bases from an interior chunk (exact everywhere with the
    # extended-knot convention fx[k] = f[clip(k-1, 0, 63)])
    c0 = 4
    Bp = np.zeros((5, CH))
    Benv = np.zeros((4, CH))
    Bp[0] = 1.0
    for j in range(4):
        u_j = U[2 * c0 - 1 + j, CH * c0: CH * (c0 + 1)]
        Benv[j] = u_j
        Bp[1 + j] = np.cumsum(u_j)
    bh = Bp.astype(np.float16)
    bl = (Bp - bh.astype(np.float64)).astype(np.float16)
    Bp = np.ascontiguousarray(np.concatenate([bh, bh, bl], axis=0))  # (15, 512)
    Benv = np.ascontiguousarray(Benv.astype(np.float16))

    # forward rDFT (t-major), bins 0..127 only; Hann window is pre-applied to
    # the noise frames on the host
    t = np.arange(WS)
    f = np.arange(NBINS)
    ang = 2.0 * np.pi * np.outer(t, f) / WS
    CwRe = np.ascontiguousarray(np.cos(ang).astype(np.float16))
    CwIm = np.ascontiguousarray((-np.sin(ang)).astype(np.float16))

    # inverse rDFT bases (OLA-fused: cols 0:256 -> y[j], cols 256:512 -> y[j+256])
    wgt = np.full(NBINS, 2.0)
    wgt[0] = 1.0
    ang2 = 2.0 * np.pi * np.outer(f, t) / WS
    Dre = np.ascontiguousarray((wgt[:, None] * np.cos(ang2) / WS).astype(np.float16))
    Dim = np.ascontiguousarray((-wgt[:, None] * np.sin(ang2) / WS).astype(np.float16))

    freqcol = (np.arange(NBINS, dtype=np.float32) / 256.0).reshape(NBINS, 1).copy()

    # within-segment interp weight profile for a 256-sample z-slice:
    # j < 128 -> (j+128.5)/256 (right knot weight), j >= 128 -> (j-127.5)/256
    j = np.arange(256)
    w1 = np.where(j < 128, (j + 128.5) / 256.0, (j - 127.5) / 256.0)
    w1bc = np.ascontiguousarray(
        np.broadcast_to(w1.astype(np.float16), (128, 256)).copy())
    w0bc = np.ascontiguousarray(
        np.broadcast_to((1.0 - w1).astype(np.float16), (128, 256)).copy())

    consts = dict(Bp=Bp, Benv=Benv, CwRe=CwRe, CwIm=CwIm,
                  Dre=Dre, Dim=Dim, freqcol=freqcol, w1bc=w1bc, w0bc=w0bc)
    _cache["consts"] = consts
    return consts


def _host_prep(inputs):
    """Vectorized host prep over all cores. Returns per-core input maps."""
    f0 = np.asarray(inputs["f0"], np.float32)
    overall_env = np.asarray(inputs["overall_env"], np.float32)
    osc_env = np.asarray(inputs["osc_env"], np.float32)
    harm_env = np.asarray(inputs["harm_env"], np.float32)
    noise_std = np.asarray(inputs["noise_std"], np.float32)
    f0b = np.asarray(inputs["f0_baselines"], np.float32)
    noise_frames = np.asarray(inputs["noise_frames"], np.float32)

    U = _interp_u()

    # match the reference's fp32 frequency values exactly: f0n and f0n*h are
    # rounded to fp32 there, and the phase trajectory follows those values
    f0c = np.clip(f0, -0.5, 0.5)
    erb = ((0.108 * (f0b * NYQ) + 24.7) / NYQ).astype(np.float32)
    f0v = np.clip(f0b + f0c * erb, 0.0, 1.0).astype(np.float32)
    f0n32 = (MIN_F0 + f0v * F0_DIFF).astype(np.float32)          # (B,16,64)
    hfact32 = np.arange(2, 2 + NH, dtype=np.float32)
    harm32 = (f0n32[:, :, None, :] * hfact32[None, None, :, None]).astype(np.float32)
    rows32 = np.concatenate([f0n32[:, :, None, :], harm32], axis=2)  # (B,16,33,64)
    fT = rows32.astype(np.float64).reshape(B, NROW, SEQ) * 0.5   # turns/sample
    f0n = f0n32.astype(np.float64)

    # chunk prefixes P[g, c] = phase before chunk c (fp64 exact), frac-reduced
    V = np.cumsum(U, axis=1)
    Vc = V[:, [CH * c - 1 for c in range(1, NCH)]]               # (64, 31)
    P = np.zeros((B, NROW, NCH))
    P[:, :, 1:] = fT @ Vc

    # extended knots fx[:, :, kk] = f[:, :, clip(kk-1, 0, 63)], kk = 0..65
    ext = np.clip(np.arange(SEQ + 2) - 1, 0, SEQ - 1)

    fx_pad = np.zeros((B, NBLK * 128, SEQ + 2), np.float64)
    fx_pad[:, :NROW] = fT[:, :, ext]
    Pf_pad = np.zeros((B, NBLK * 128, NCH), np.float64)
    Pf_pad[:, :NROW] = np.mod(P, 1.0)

    # phase coefs frc[b][15, NCH*NBLK*128]: hi/lo fp16 split [ch; cl; ch],
    # free index = (c*NBLK + blk)*128 + r
    frc = np.zeros((B, NCH, NBLK, 5, 128), np.float64)
    for c in range(NCH):
        blkv = fx_pad[:, :, 2 * c: 2 * c + 4]                    # (B,640,4)
        kn = blkv.reshape(B, NBLK, 128, 4).transpose(0, 1, 3, 2)  # (B,5,4,128)
        frc[:, c, :, 1:5, :] = kn
        frc[:, c, :, 0, :] = Pf_pad[:, :, c].reshape(B, NBLK, 128)
    ch = frc.astype(np.float16)
    cl = (frc - ch.astype(np.float64)).astype(np.float16)
    frc = np.concatenate([ch, cl, ch], axis=3)                   # (B,NCH,NBLK,15,128)
    frc = np.ascontiguousarray(
        frc.transpose(0, 3, 1, 2, 4).reshape(B, 15, NCH * NBLK * 128))

    # env-valued (negated) selection weights per extended knot kk and block b:
    # layout [128, 16 + (kk*NBLK+b)*32 ...]: 16 leading zeros, then per index
    # i = kk*NBLK+b a [16 w-cols | 16 zero-cols] pair. The u=0 stationary
    # slice is cols [16+32i, 16+32i+32) = [w | 0]; u=1 is [32i, 32i+32) = [0 | w].
    oe = np.clip(osc_env, 0.0, 1.0).astype(np.float32)
    he = np.clip(harm_env, 0.0, 1.0).astype(np.float32)
    env_rows = oe[:, :, None, :] * np.concatenate(
        [np.ones((B, NE, 1, SEQ), np.float32), he], axis=2)
    env_rows = env_rows.reshape(B, NROW, SEQ)
    NKK = SEQ + 2
    evw = np.zeros((B, NKK, NBLK, 128, 16), np.float16)
    gidx = np.arange(NROW)
    ev = gidx // 33
    blk = gidx // 128
    rr = gidx % 128
    envx = -env_rows[:, :, ext]                                  # (B,528,66)
    evw[:, :, blk, rr, ev] = envx[:, gidx].transpose(0, 2, 1)
    ewpad = np.zeros((B, NKK, NBLK, 128, 32), np.float16)
    ewpad[..., 0:16] = evw
    ew = np.zeros((B, 128, 16 + NKK * NBLK * 32), np.float16)
    ew[:, :, 16:] = ewpad.transpose(0, 3, 1, 2, 4).reshape(B, 128, NKK * NBLK * 32)
    ew = np.ascontiguousarray(ew)

    # overall mix coefs, u-split: ovc[b][4, (c*2+u)*32 + row], zero outside
    # the u-th 16-row group
    ov = np.clip(overall_env, 0.0, 1.0).astype(np.float32)
    ovx = ov[:, :, ext]                                          # (B,16,66)
    ovc = np.zeros((B, NCH, 2, 4, 32), np.float16)
    for c in range(NCH):
        k = ovx[:, :, 2 * c: 2 * c + 4].transpose(0, 2, 1)       # (B,4,16)
        ovc[:, c, 0, :, 0:16] = k
        ovc[:, c, 1, :, 16:32] = k
    ovc = np.ascontiguousarray(
        ovc.transpose(0, 3, 1, 2, 4).reshape(B, 4, NCH * 64))

    # noise: mean/c2 rows frame-major (fr = s*16 + e)
    std = (np.clip(noise_std, 1e-12, 1.0) * F0_DIFF).astype(np.float32)
    c2 = (-0.5 / (std.astype(np.float64) ** 2)).astype(np.float32)
    f0n32 = f0n.astype(np.float32)
    mcrow = np.ascontiguousarray(
        np.stack([f0n32.transpose(0, 2, 1).reshape(B, NFR),
                  c2.transpose(0, 2, 1).reshape(B, NFR)], axis=1))

    # pre-windowed, transposed noise frames: nfT[b][t, fr] fp16
    t = np.arange(WS)
    win = (0.5 - 0.5 * np.cos(2.0 * np.pi * t / WS)).astype(np.float32)
    nf = noise_frames.transpose(0, 2, 1, 3).reshape(B, NFR, WS)  # (B, fr, t)
    nfT = np.ascontiguousarray(
        (nf * win[None, None, :]).transpose(0, 2, 1).astype(np.float16))

    consts = _build_consts()
    in_maps = []
    for b in range(B):
        m = dict(frc=frc[b], ew=ew[b], ovc=ovc[b], mcrow=mcrow[b], nfT=nfT[b])
        m.update(consts)
        in_maps.append(m)
    return in_maps


def _build_nc():
    if "nc" in _cache:
        return _cache["nc"]
    from concourse import bacc
    import concourse.tile as tile
    from concourse import mybir
    from contextlib import ExitStack

    F32 = mybir.dt.float32
    F32R = mybir.dt.float32r
    FP16 = mybir.dt.float16
    AF = mybir.ActivationFunctionType
    OP = mybir.AluOpType

    nc = bacc.Bacc()
    frc = nc.declare_dram_parameter("frc", [15, NCH * NBLK * 128], FP16, isOutput=False)
    ew = nc.declare_dram_parameter("ew", [128, 16 + (SEQ + 2) * NBLK * 32], FP16, isOutput=False)
    ovc = nc.declare_dram_parameter("ovc", [4, NCH * 64], FP16, isOutput=False)
    mcrow = nc.declare_dram_parameter("mcrow", [2, NFR], F32, isOutput=False)
    nfT = nc.declare_dram_parameter("nfT", [WS, NFR], FP16, isOutput=False)
    Bp = nc.declare_dram_parameter("Bp", [15, CH], FP16, isOutput=False)
    Benv = nc.declare_dram_parameter("Benv", [4, CH], FP16, isOutput=False)
    CwRe = nc.declare_dram_parameter("CwRe", [WS, NBINS], FP16, isOutput=False)
    CwIm = nc.declare_dram_parameter("CwIm", [WS, NBINS], FP16, isOutput=False)
    Dre = nc.declare_dram_parameter("Dre", [NBINS, WS], FP16, isOutput=False)
    Dim = nc.declare_dram_parameter("Dim", [NBINS, WS], FP16, isOutput=False)
    freqcol = nc.declare_dram_parameter("freqcol", [NBINS, 1], F32, isOutput=False)
    w1bc = nc.declare_dram_parameter("w1bc", [128, 256], FP16, isOutput=False)
    w0bc = nc.declare_dram_parameter("w0bc", [128, 256], FP16, isOutput=False)
    out = nc.declare_dram_parameter("out", [NE, N], F32, isOutput=True)

    CB = 2 * NBLK * 128   # coef columns per c2 iteration (1280)

    with tile.TileContext(nc) as tc, ExitStack() as ctx:
        cp = ctx.enter_context(tc.tile_pool(name="const", bufs=1))

        # noise inputs first so the noise pipeline starts ASAP
        mr_sb = cp.tile([1, NFR], F32, tag="mr")
        nc.sync.dma_start(mr_sb[:], mcrow[0:1, :])
        c2r_sb = cp.tile([1, NFR], F32, tag="c2r")
        nc.sync.dma_start(c2r_sb[:], mcrow[1:2, :])
        cwre_sb = cp.tile([128, 4 * NBINS], FP16, tag="cwre")
        cwim_sb = cp.tile([128, 4 * NBINS], FP16, tag="cwim")
        for t4 in range(4):
            nc.sync.dma_start(cwre_sb[:, t4 * NBINS:(t4 + 1) * NBINS],
                              CwRe[t4 * 128:(t4 + 1) * 128, :])
            nc.sync.dma_start(cwim_sb[:, t4 * NBINS:(t4 + 1) * NBINS],
                              CwIm[t4 * 128:(t4 + 1) * 128, :])
        nft_sb = [cp.tile([128, NFR], FP16, tag=f"nft{t4}", name=f"nft{t4}")
                  for t4 in range(4)]
        for t4 in range(4):
            nc.sync.dma_start(nft_sb[t4][:], nfT[t4 * 128:(t4 + 1) * 128, :])
        freqcol_sb = cp.tile([NBINS, 1], F32, tag="freqcol")
        nc.sync.dma_start(freqcol_sb[:], freqcol[:])
        dre_sb = cp.tile([NBINS, WS], FP16, tag="dre")
        nc.sync.dma_start(dre_sb[:], Dre[:])
        dim_sb = cp.tile([NBINS, WS], FP16, tag="dim")
        nc.sync.dma_start(dim_sb[:], Dim[:])

        # osc constants
        bp_sb = cp.tile([15, CH], FP16, tag="bp")
        nc.sync.dma_start(bp_sb[:], Bp[:])
        benv_sb = cp.tile([4, CH], FP16, tag="benv")
        nc.sync.dma_start(benv_sb[:], Benv[:])
        w1_sb = cp.tile([128, 256], FP16, tag="w1bc")
        nc.sync.dma_start(w1_sb[:], w1bc[:])
        w0_sb = cp.tile([128, 256], FP16, tag="w0bc")
        nc.sync.dma_start(w0_sb[:], w0bc[:])
        ew_sb = cp.tile([128, 16 + (SEQ + 2) * NBLK * 32], FP16, tag="ew")
        nc.sync.dma_start(ew_sb[:], ew[:])
        ovc_sb = cp.tile([4, NCH * 64], FP16, tag="ovc")
        nc.sync.dma_start(ovc_sb[:], ovc[:])
        bsin = cp.tile([128, 1], F32, tag="bsin")
        nc.vector.memset(bsin[:], 0.0)
        b1024 = cp.tile([128, 1], F32, tag="b1024")
        nc.vector.memset(b1024[:], 1024.0)

        # ---------------- noise phase (PSUM pools scoped) ----------------
        na = ctx.enter_context(tc.tile_pool(name="na", bufs=1))
        nzS = [na.tile([128, 256], FP16, tag=f"nz{g}", name=f"nz{g}")
               for g in range(NGRP)]
        with tc.tile_pool(name="psN", bufs=2, space="PSUM") as psN, \
             tc.tile_pool(name="psZ", bufs=2, space="PSUM") as psZ:
            mean_bc = na.tile([128, NFR], F32, tag="meanbc")
            nc.gpsimd.partition_broadcast(mean_bc[:], mr_sb[:])
            c2_bc = na.tile([128, NFR], F32, tag="c2bc")
            nc.gpsimd.partition_broadcast(c2_bc[:], c2r_sb[:])
            # filt = exp(c2 * (freq - mean)^2)
            fb = na.tile([128, NFR], F32, tag="fb")
            nc.scalar.activation(fb[:], mean_bc[:], AF.Square,
                                 bias=freqcol_sb[:], scale=-1.0)
            fm = na.tile([128, NFR], F32, tag="fm")
            nc.gpsimd.tensor_tensor(fm[:], fb[:], c2_bc[:], OP.mult)
            ff = na.tile([128, NFR], FP16, tag="ff")
            nc.scalar.activation(ff[:], fm[:], AF.Exp)

            # rfft (bins 0..127) + filter; specf tiles have 16 zero lead cols
            sfre = na.tile([128, 16 + NFR], FP16, tag="sfre")
            sfim = na.tile([128, 16 + NFR], FP16, tag="sfim")
            nc.vector.memset(sfre[:, 0:16], 0.0)
            nc.vector.memset(sfim[:, 0:16], 0.0)
            for h in range(2):
                sl = slice(h * 512, (h + 1) * 512)
                spr = psN.tile([128, 512], F32, tag="spr")
                spi = psN.tile([128, 512], F32, tag="spi")
                for t4 in range(4):
                    nc.tensor.matmul(spr[:], cwre_sb[:, t4 * NBINS:(t4 + 1) * NBINS],
                                     nft_sb[t4][:, sl], start=(t4 == 0), stop=(t4 == 3))
                for t4 in range(4):
                    nc.tensor.matmul(spi[:], cwim_sb[:, t4 * NBINS:(t4 + 1) * NBINS],
                                     nft_sb[t4][:, sl], start=(t4 == 0), stop=(t4 == 3))
                nc.vector.tensor_tensor(sfre[:, 16 + h * 512:16 + (h + 1) * 512],
                                        spr[:], ff[:, sl], OP.mult)
                nc.vector.tensor_tensor(sfim[:, 16 + h * 512:16 + (h + 1) * 512],
                                        spi[:], ff[:, sl], OP.mult)

            # inverse rDFT + OLA fused; nzS[g] in z-major [16z+e, 256j] fp16
            for g in range(NGRP):
                nzp = psZ.tile([128, 256], F32, tag="nzp")
                nc.tensor.matmul(nzp[:], sfre[:, 16 + g * 128: 16 + g * 128 + 128],
                                 dre_sb[:, 0:256], start=True, stop=False)
                nc.tensor.matmul(nzp[:], sfim[:, 16 + g * 128: 16 + g * 128 + 128],
                                 dim_sb[:, 0:256], start=False, stop=False)
                nc.tensor.matmul(nzp[:], sfre[:, g * 128: g * 128 + 128],
                                 dre_sb[:, 256:512], start=False, stop=False)
                nc.tensor.matmul(nzp[:], sfim[:, g * 128: g * 128 + 128],
                                 dim_sb[:, 256:512], start=False, stop=True)
                nc.scalar.copy(nzS[g][:], nzp[:])

        # ---------------- oscillator phase ----------------
        cof = ctx.enter_context(tc.tile_pool(name="cof", bufs=3))
        dsp = ctx.enter_context(tc.tile_pool(name="dsp", bufs=8))
        stp = ctx.enter_context(tc.tile_pool(name="stp", bufs=12))
        ocp = ctx.enter_context(tc.tile_pool(name="ocp", bufs=2))
        psA = ctx.enter_context(tc.tile_pool(name="psA", bufs=2, space="PSUM"))
        psG = ctx.enter_context(tc.tile_pool(name="psG", bufs=2, space="PSUM"))
        psM = ctx.enter_context(tc.tile_pool(name="psM", bufs=2, space="PSUM"))

        def ew_sl(kk, b, u):
            i = kk * NBLK + b
            base = 16 + 32 * i if u == 0 else 32 * i
            return ew_sb[:, base: base + 32]

        gbans = {}
        pmbans = {}

        def emit_front(c2):
            """Phase matmuls + range reduction + sin for iteration c2."""
            fc_sb = cof.tile([15, CB], FP16, tag="fc")
            nc.sync.dma_start(fc_sb[:], frc[:, c2 * CB:(c2 + 1) * CB])
            sts = []
            for b in range(NBLK):
                pa = psA.tile([128, 1024], F32, tag="pa")
                for ci in range(2):
                    idx = (ci * NBLK + b) * 128
                    nc.tensor.matmul(pa[:, ci * 512:(ci + 1) * 512],
                                     fc_sb[:, idx:idx + 128],
                                     bp_sb[:], start=True, stop=True)
                # range reduction: yt = fp16(phase + 1024) rounds to the
                # nearest integer (fp16 ulp is exactly 1 in [1024, 2048));
                # then -dt = (yt - 1024) - phase via scalar_tensor_tensor.
                yt = dsp.tile([128, 1024], FP16, tag="yt")
                if b < 3:
                    nc.scalar.activation(yt[:], pa[:], AF.Identity,
                                         bias=b1024[:], scale=1.0)
                else:
                    nc.vector.tensor_scalar(yt[:], pa[:], 1024.0, None, OP.add)
                dt_ = dsp.tile([128, 1024], FP16, tag="dt")
                nc.vector.scalar_tensor_tensor(dt_[:], yt[:], 1024.0, pa[:],
                                               OP.subtract, OP.subtract)
                st = stp.tile([128, 1024], FP16, tag="st")
                nc.scalar.activation(st[:], dt_[:], AF.Sin, bias=bsin[:],
                                     scale=SIN_SCALE)
                sts.append((st, 0))
            return sts

        def emit_back(c2, sts):
            cA = 2 * c2
            grp = cA // 4
            if cA % 4 == 0:
                gbans[grp] = psG.tile([128, 512], F32, tag="gb", name=f"gb{grp}")
                pmbans[grp] = psM.tile([128, 256], F32, tag="pm", name=f"pm{grp}")
            gb = gbans[grp]
            pm = pmbans[grp]

            # env-weighted selection matmuls: Glo (gb cols 0:256) holds the
            # left-knot weighted harmonic sum, Ghi (cols 256:512) the right
            for ci in range(2):
                cc = cA + ci
                zp = cc % 4
                for u in range(2):
                    mkk = 2 * cc + u
                    for b in range(NBLK):
                        stt_, base = sts[b]
                        o0 = base + ci * 512 + u * 256
                        stA = stt_[:, o0: o0 + 128]
                        stB = stt_[:, o0 + 128: o0 + 256]
                        # start=True only on the very first matmul touching this
                        # 32-row bank region: its start marks the whole zero
                        # region pending, and first-touch zeroing initializes
                        # the other column ranges
                        fb_ = (u == 0 and b == 0)
                        lb_ = (u == 1 and b == NBLK - 1)
                        nc.tensor.matmul(gb[32 * zp:32 * zp + 32, 0:128],
                                         ew_sl(mkk, b, u), stA,
                                         start=fb_, stop=False,
                                         skip_group_check=True, tile_position=(0, 32 * zp))
                        nc.tensor.matmul(gb[32 * zp:32 * zp + 32, 128:256],
                                         ew_sl(mkk + 1, b, u), stB,
                                         start=False, stop=False,
                                         skip_group_check=True, tile_position=(0, 32 * zp))
                        nc.tensor.matmul(gb[32 * zp:32 * zp + 32, 256:384],
                                         ew_sl(mkk + 1, b, u), stA,
                                         start=False, stop=False,
                                         skip_group_check=True, tile_position=(0, 32 * zp))
                        nc.tensor.matmul(gb[32 * zp:32 * zp + 32, 384:512],
                                         ew_sl(mkk + 2, b, u), stB,
                                         start=False, stop=lb_,
                                         skip_group_check=True, tile_position=(0, 32 * zp))
                # mix interp into pm (z-major rows)
                for u in range(2):
                    nc.tensor.matmul(
                        pm[32 * zp:32 * zp + 32, 0:256],
                        ovc_sb[:, (cc * 2 + u) * 32:(cc * 2 + u + 1) * 32],
                        benv_sb[:, u * 256:(u + 1) * 256],
                        start=(u == 0), stop=(u == 1), skip_group_check=True,
                        tile_position=(0, 32 * zp))

            if cA % 4 == 2:
                # blend: posc = w0*Glo + w1*Ghi (each op reads <=1 PSUM input)
                glo = gb[:, 0:256]
                ghi = gb[:, 256:512]
                t1 = ocp.tile([128, 256], FP16, tag="t1")
                nc.vector.tensor_tensor(t1[:], glo, w0_sb[:], OP.mult)
                t2 = ocp.tile([128, 256], FP16, tag="t2")
                nc.vector.tensor_tensor(t2[:], ghi, w1_sb[:], OP.mult)
                posc_s = ocp.tile([128, 256], F32, tag="ps")
                nc.vector.tensor_tensor(posc_s[:], t1[:], t2[:], OP.add)
                pc = ocp.tile([128, 256], FP16, tag="pc")
                nc.scalar.copy(pc[:], pm[:])
                av = ocp.tile([128, 256], F32, tag="av")
                nc.gpsimd.tensor_tensor(av[:], posc_s[:], nzS[grp][:], OP.subtract)
                bv = ocp.tile([128, 256], F32, tag="bv")
                nc.gpsimd.tensor_tensor(bv[:], av[:], pc[:], OP.mult)
                ov_ = ocp.tile([128, 256], F32, tag="ov")
                nc.gpsimd.tensor_tensor(ov_[:], bv[:], nzS[grp][:], OP.add)
                for z in range(8):
                    nc.sync.dma_start(
                        out[:, grp * 2048 + z * 256: grp * 2048 + (z + 1) * 256],
                        ov_[16 * z:16 * (z + 1), :])
                del gbans[grp]
                del pmbans[grp]

        # software-pipelined emission: the back half (selection matmuls and
        # combine) lags one iteration behind the front half (phase/sin), so
        # the PE never stalls waiting on freshly produced sins
        pending = None
        for c2 in range(NCH // 2):
            sts_new = emit_front(c2)
            if pending is not None:
                emit_back(*pending)
            pending = (c2, sts_new)
        emit_back(*pending)

    nc.finalize()
    _cache["nc"] = nc
    return nc


def kernel(**inputs):
    from concourse.bass_utils import run_bass_kernel_spmd

    in_maps = _host_prep(inputs)
    nc = _build_nc()

    trace = bool(os.environ.get("BASS_PROFILE"))
    res = run_bass_kernel_spmd(nc, in_maps, list(range(B)), trace=trace)
    if trace and res.exec_time_ns is not None:
        print(f"HW exec time: {res.exec_time_ns} ns")
    out = np.stack([r["out"] for r in res.results]).astype(np.float32)
    return out
